# revision 61
# baseline (speedup 1.0000x reference)
"""Trainium2 Bass kernel for nn_MultiHeadAttention_3762391351798.

Takes FULL inputs, returns the FULL output. Internally shards across 8
NeuronCores: data-parallel over batch (B=4) x tensor-parallel over head
halves (2 groups of 8 heads). Per core (batch b, head-group g):

  Phase A/B (QKV projection):
  - Q/K projections run in fp8e4 with MatmulPerfMode.DoubleRow: weights
    host-prescaled by 32 (fp8 subnormal avoidance; folded back via the
    exp scale 2^-13), contraction pairs of 128-row subtiles -> 4x fewer
    PE cycles than fp16. V projection stays fp16 (fp8 V fails the 2e-2
    accuracy gate).
  - Q (ACT Identity+bias) / K (DVE copy) evacuate PSUM directly to fp8
    staging tiles in psum column order; SBUF->SBUF DMAs then scatter
    them into the DoubleRow slot layout qT8/kT8 [32-partition slots at
    bases 0/64][pair][d-half j][t] (DMA is the only partition-crossing
    path; 8 DMAs per half-T per section).
  - pre-run S units start inside this phase so the attention phase
    opens with a full exp pipeline.

  Phase C/D (attention + out-projection, interleaved):
  - S^T tiles computed in fp8 DoubleRow: lhsT = kT8 slot [32, 2(j), 128
    keys], rhs = qT8 slot [32, 2(j), w], out [128, w] PSUM, d = 32j+r
    contraction; exp on ACT with scale 2^-13 (absorbs the 32x32 weight
    prescale and 1/8 softmax scale)
  - causal masking of the diagonal 128x128 block via multiplicative 0/1
    fp16 mask on DVE after the exp (pt tiles stay fp16)
  - AV in fp16 with a ones column appended to V for free denominators;
    flat software pipeline (chunk, pair, key-tile) with the AV cursor a
    constant lag behind the S/exp cursor; ch3 V-projection groups are
    deferred into this phase to fill PE bubbles
  - normalization per 512-query window: DVE reciprocal -> DRAM-
    roundtrip broadcast -> DVE multiply; odd-head results cross
    partitions on the Pool engine
  - out-projection drains one tile at a time through the score-pool
    PSUM, aged past the normalization chain

Host sums the two partials per batch (the only cross-core reduction).

Math notes vs the reference: softmax is shift invariant, so the row-max
subtraction, the k-bias term and bq . bk are dropped; the q-bias IS
kept. The v-bias is folded into an effective out-bias on the host:
out = attn @ Wout + (bv @ Wout + bout).

Accuracy: fp8e4m3 Q/K projection + scores measure ~1.6e-2 relative
error end to end (gate 2e-2); V/AV/out-projection stay fp16 because
fp8 there pushes past the gate.

Hardware constraints honored: DMA and matmul instructions tolerate a
single semaphore wait (split_waits moves any excess onto standalone
event-semaphore stubs); SBUF AP base partitions must be 0/32/64 (slot
bases are 0 and 64).
"""

import numpy as np

import concourse.bass as bass
import concourse.mybir as mybir
import concourse.tile as tile
from concourse import library_config  # noqa: F401

F32 = mybir.dt.float32
F16 = mybir.dt.float16
F8 = mybir.dt.float8e4

P = 128
DR = mybir.MatmulPerfMode.DoubleRow
WSC = 32.0                      # host prescale on Wq/Wk (and bq)
SSCALE = 0.125 / (WSC * WSC)    # exp scale: folds prescale^2 + 1/sqrt(d)


def split_waits(nc, keep=1):
    """Walrus codegen rejects instructions carrying more than ~1 semaphore
    wait on several ISA structs ("Too many sync wait commands"). Move excess
    waits onto standalone InstEventSemaphore instructions on the same engine
    immediately before the original instruction (same per-engine program
    order, so semantics are unchanged)."""
    n = 0
    for bb in nc.m.functions[0].blocks:
        out = []
        for inst in bb.instructions:
            si = inst.sync_info
            if si is not None and len(si.on_wait) > keep:
                waits = list(si.on_wait)
                move, stay = waits[:-keep] if keep else waits, \
                    waits[-keep:] if keep else []
                for i, w in enumerate(move):
                    n += 1
                    out.append(mybir.InstEventSemaphore(
                        name=f"{inst.name}-sw{i}", engine=inst.engine,
                        ins=[], outs=[],
                        sync_info=mybir.SyncInfo(on_wait=[w], on_update=[])))
                inst.sync_info = mybir.SyncInfo(
                    on_wait=stay, on_update=list(si.on_update))
            out.append(inst)
        bb.instructions = out
    return n


def build_nc(T=2048, C=1024, HL=8, D=64, trace_sim=False, split=True,
             n_iters=1, drain_every=3, drain_age=6, pre_pairs=1,
             lag_max=16, qevac_act_early=False, v_in_stream=False):
    """Build the per-core Bass program (identical on all cores)."""
    CL = HL * D          # local q/k/v width (512)
    KO = C // P          # contraction subtiles over C (8)
    NT = T // P          # 128-row key tiles over T (16)
    TC = 512             # T-chunk for the projection phase
    NCH = T // TC
    CO = CL // P         # 128-col blocks per q/k section (4)
    QC = 1024            # attention query chunk
    NQC = T // QC        # 2
    NP = HL // 2         # head pairs (4)
    EXP = mybir.ActivationFunctionType.Exp
    IDENT = mybir.ActivationFunctionType.Identity

    nc = bass.Bass(target_bir_lowering=False, debug=False)

    x8_d = nc.dram_tensor("x8", [C, T], F8, kind="ExternalInput").ap()
    x_d = nc.dram_tensor("x", [C, T], F16, kind="ExternalInput").ap()
    w8_d = nc.dram_tensor("wqk8", [C, 2 * CL], F8, kind="ExternalInput").ap()
    wv_d = nc.dram_tensor("wv", [C, CL], F16, kind="ExternalInput").ap()
    bq_d = nc.dram_tensor("bq", [CL], F32, kind="ExternalInput").ap()
    wr_d = nc.dram_tensor("wout", [CL, C], F16, kind="ExternalInput").ap()
    bout_d = nc.dram_tensor("bout", [C], F32, kind="ExternalInput").ap()
    out_d = nc.dram_tensor("out", [T, C], F32, kind="ExternalOutput").ap()

    with tile.TileContext(nc, trace_sim=trace_sim) as tc:
        with (
            tc.tile_pool(name="const", bufs=1) as const_pool,
            tc.tile_pool(name="persist", bufs=1) as persist,
            tc.tile_pool(name="dram", bufs=64, space="DRAM") as dram_pool,
        ):
            bq_sb = const_pool.tile([P, CO], F32)
            # multiplicative causal mask for the diagonal 128x128 block:
            # trimask[r, c] = 1 if c >= r else 0 (row 0 doubles as an
            # all-ones row for the bias-seeding matmul in tail jobs);
            # trimask2 duplicates it along a middle dim so one DVE op can
            # mask both heads of a merged [P, 2, P] exp tile
            trimask = const_pool.tile([P, P], F16)
            nc.gpsimd.memset(trimask, 1.0)
            nc.gpsimd.affine_select(
                out=trimask, in_=trimask, compare_op=mybir.AluOpType.is_ge,
                fill=0.0, base=0, pattern=[[1, P]], channel_multiplier=-1)
            trimask2 = const_pool.tile([P, 2, P], F16)
            nc.gpsimd.memset(trimask2, 1.0)
            nc.gpsimd.affine_select(
                out=trimask2, in_=trimask2, compare_op=mybir.AluOpType.is_ge,
                fill=0.0, base=0, pattern=[[0, 2], [1, P]],
                channel_multiplier=-1)

            # out-projection weights / bias (transfers emitted later so they
            # do not contend with the W/x loads on the DMA engines)
            wr_sb = const_pool.tile([P, CO, C], F16)
            bout_b = const_pool.tile([P, C], F32)
            bout_row = const_pool.tile([1, C], F16)
            nc.gpsimd.dma_start(bout_row, bout_d[None, :])

            # fp8 DoubleRow slot layout: head h = 2*pi + h2 lives at
            # partitions 64*h2 .. 64*h2+31, dim1 = d-half j (d = 32*j+r),
            # dim2 = pi, dim3 = t. j outermost-after-partition so a single
            # DMA per (stage, h2) scatters the j-interleaved staging
            qT8 = persist.tile([P, 2, NP, T], F8)
            kT8 = persist.tile([P, 2, NP, T], F8)
            vt = persist.tile([P, NT, HL, D + 1], F16)   # [V | ones]
            chunkT = persist.tile([P, CO, T], F16)

            nc.gpsimd.memset(vt[:, :, :, D:D + 1], 1.0)

            for _it in range(n_iters):  # >1 only for benchmarking
                # score/exp pools span both phases (for the pre-run units)
                lagm = lag_max if lag_max is not None else 8 * pre_pairs
                with (
                    tc.tile_pool(name="ps", bufs=2, space="PSUM") as ps_psum,
                    tc.tile_pool(name="pt", bufs=lagm + 4) as pt_pool,
                    tc.tile_pool(name="wqo", bufs=1) as wq_outer,
                ):
                    wv_sb = wq_outer.tile([P, KO, CL], F16)
                    x16 = wq_outer.tile([P, KO, T], F16)
                    ptts = {}   # (c0, pi, kt) -> merged exp'd score tile

                    def emit_s_unit(u):
                        # both heads of the pair share one [P, 2, 512] psum
                        # tile per 512-query half, so a single exp (and a
                        # single diag-mask multiply) covers the pair
                        c0, pi, kt = u
                        qlo = c0 * QC
                        qstart = max(qlo, kt * P)
                        w = qlo + QC - qstart
                        diag = kt * P >= qlo
                        big = w > 512
                        pt = pt_pool.tile(
                            [P, 2, QC] if big else [P, 2, 512], F16,
                            tag="pt" if big else "pth",
                            bufs=9 if big else 6)
                        for half in range(0, w, 512):
                            hw = min(512, w - half)
                            pst = ps_psum.tile([P, 2, 512], F32, tag="ps")
                            for h2 in range(2):
                                hb = 64 * h2
                                nc.tensor.matmul(
                                    pst[:, h2, 0:hw],
                                    lhsT=kT8[hb:hb + 32, :, pi,
                                             kt * P:(kt + 1) * P],
                                    rhs=qT8[hb:hb + 32, :, pi,
                                            qstart + half:qstart + half + hw],
                                    start=True, stop=True, perf_mode=DR)
                            nc.scalar.activation(
                                pt[:, :, half:half + hw], pst[:, :, 0:hw],
                                EXP, scale=SSCALE)
                            if diag and half == 0:
                                nc.vector.tensor_tensor(
                                    pt[:, :, 0:P], pt[:, :, 0:P], trimask2,
                                    mybir.AluOpType.mult)
                        ptts[u] = pt

                    # 12 pre-run units (pair 0 fully + pair 1 kt 0..3):
                    # their exps cover the V-projection block, where PE is
                    # busy but no new S tiles are produced
                    pre_units = ([(0, 0, kt) for kt in range(QC // P)]
                                 + [(0, 1, kt) for kt in range(4)])
                    pre_iter = iter(pre_units)

                    # ------------- Phase A/B: QKV projection -----------------
                    with (
                        tc.tile_pool(name="xf", bufs=1) as xf_pool,
                        tc.tile_pool(name="pp", bufs=3, space="PSUM")
                        as pp_psum,
                    ):
                        w8_sb = xf_pool.tile([P, KO, 2 * CL], F8)
                        x8full = xf_pool.tile([P, KO, T], F8)
                        # psum-column-order fp8 staging for Q/K before the
                        # slot-scatter DMAs
                        qstage = xf_pool.tile([P, CO, T], F8)
                        kstage = xf_pool.tile([P, CO, T], F8)

                        x8re = x8_d.rearrange("(o p) t -> p o t", p=P)
                        xre = x_d.rearrange("(o p) t -> p o t", p=P)
                        w8re = w8_d.rearrange("(o p) c -> p o c", p=P)
                        wvre = wv_d.rearrange("(o p) c -> p o c", p=P)

                        # queue discipline: the scalar (ACT) HWDGE queue
                        # carries only the small early loads (bq/w8/wv, all
                        # dispatched by ~5us) so the ACT sequencer is free
                        # once the exp stream starts; bulk x rides the SP
                        # queue; k-bounces + out-proj weights ride the Pool
                        # SWDGE queue. Order = first use, so the shared DMA
                        # engines drain in priority order.
                        if _it == 0:
                            nc.scalar.dma_start(
                                bq_sb, bq_d.rearrange("(o p) -> p o", p=P))
                        nc.sync.dma_start(x8full[:, :, 0:TC],
                                          x8re[:, :, 0:TC])
                        nc.scalar.dma_start(w8_sb[:, :, 0:CL],
                                            w8re[:, :, 0:CL])
                        nc.scalar.dma_start(w8_sb[:, :, CL:2 * CL],
                                            w8re[:, :, CL:2 * CL])
                        nc.sync.dma_start(x8full[:, :, TC:2 * TC],
                                          x8re[:, :, TC:2 * TC])
                        nc.scalar.dma_start(x8full[:, :, 2 * TC:4 * TC],
                                            x8re[:, :, 2 * TC:4 * TC])
                        for kh in range(2):
                            nc.scalar.dma_start(
                                wv_sb[:, 4 * kh:4 * kh + 4, :],
                                wvre[:, 4 * kh:4 * kh + 4, :])
                        # the x16 chunks and out-proj weights ride the
                        # in-order scalar queue so the scheduler cannot
                        # hoist them ahead of the critical w8k transfer
                        # (SP's blocked bounce DMAs don't pin order)
                        for ch in range(NCH):
                            nc.scalar.dma_start(
                                x16[:, :, ch * TC:(ch + 1) * TC],
                                xre[:, :, ch * TC:(ch + 1) * TC])
                        wrre = wr_d.rearrange("(o p) c -> p o c", p=P)
                        for kh in range(2):
                            nc.scalar.dma_start(
                                wr_sb[:, 2 * kh:2 * kh + 2, :],
                                wrre[:, 2 * kh:2 * kh + 2, :])
                        nc.scalar.dma_start(
                            bout_b, bout_d[None, :].to_broadcast((P, C)))

                        # PE pstate warmup: ~3.5us of dependency-free
                        # matmuls on the trimask constant while the x8/w8
                        # loads land, so the projection opens at full clock
                        for wi_ in range(7):
                            pw = pp_psum.tile([P, 512], F32, tag="pp")
                            for _h in range(4):
                                nc.tensor.matmul(
                                    pw[:, _h * P:(_h + 1) * P],
                                    lhsT=trimask, rhs=trimask,
                                    start=True, stop=True)

                        def qk_group(sec, co, ch):
                            # fp8 DoubleRow: contraction = 4 pairs of
                            # 128-row subtiles
                            pp = pp_psum.tile([P, TC], F32, tag="pp")
                            for m in range(KO // 2):
                                nc.tensor.matmul(
                                    pp,
                                    lhsT=w8_sb[:, 2 * m:2 * m + 2,
                                               sec * CL + co * P:
                                               sec * CL + (co + 1) * P],
                                    rhs=x8full[:, 2 * m:2 * m + 2,
                                               ch * TC:(ch + 1) * TC],
                                    start=(m == 0), stop=(m == KO // 2 - 1),
                                    perf_mode=DR)
                            # Q evacs on ACT (idle during the projection
                            # head; bias comes free via Identity), K on
                            # DVE: the two evac chains run concurrently.
                            # Only DVE/ACT may read PSUM on HW.
                            stage = qstage if sec == 0 else kstage
                            dst = stage[:, co, ch * TC:(ch + 1) * TC]
                            if sec == 0 and ch < 2 and qevac_act_early:
                                nc.scalar.activation(
                                    dst, pp, IDENT,
                                    bias=bq_sb[:, co:co + 1])
                            elif sec == 0:
                                nc.vector.tensor_scalar_add(
                                    dst, pp, bq_sb[:, co:co + 1])
                            else:
                                nc.vector.tensor_copy(dst, pp)

                        def bounce(c0_, c1_, t0, t1):
                            # scatter the staging tiles into the DoubleRow
                            # slot layout. The host permutes W columns so
                            # stage partition 64*h2 + 2*r + j holds head
                            # 2*co+h2, d = 32*j + r: one DMA per (stage,
                            # h2) then iterates (r, j, co, t) on both
                            # sides. SP HWDGE queue: dispatch does not
                            # occupy a compute engine.
                            for stage, dstt in ((qstage, qT8),
                                                (kstage, kT8)):
                                for h2 in range(2):
                                    nc.sync.dma_start(
                                        dstt[64 * h2:64 * h2 + 32, :,
                                             c0_:c1_, t0:t1],
                                        stage[64 * h2:64 * h2 + 64,
                                              c0_:c1_, t0:t1])

                        def vjob0(kt_idx):
                            pv = pp_psum.tile([P, CL], F32, tag="pp")
                            for ko in range(KO):
                                nc.tensor.matmul(
                                    pv,
                                    lhsT=x16[:, ko,
                                             kt_idx * P:(kt_idx + 1) * P],
                                    rhs=wv_sb[:, ko, :],
                                    start=(ko == 0), stop=(ko == KO - 1),
                                )
                            nc.vector.tensor_copy(
                                vt[:, kt_idx, :, 0:D],
                                pv.rearrange("p (h d) -> p h d", d=D))

                        # first T-half co-major: co 0 of both chunks and
                        # both sections first, then its bounce, so the
                        # pair-0 pre-run S units can start ~9us in
                        for co in range(CO):
                            for sec in range(2):
                                for ch in range(2):
                                    qk_group(sec, co, ch)
                            bounce(co, co + 1, 0, 2 * TC)
                            if co > 0:
                                u = next(pre_iter, None)
                                if u is not None:
                                    emit_s_unit(u)
                        # second T-half
                        for ch in range(2, NCH):
                            for sec in range(2):
                                for co in range(CO):
                                    qk_group(sec, co, ch)
                                    u = next(pre_iter, None)
                                    if u is not None:
                                        emit_s_unit(u)
                        bounce(0, CO, 2 * TC, 4 * TC)

                        # V projection (fp16): chunk 0 always here (first
                        # AV units need it); chunks 1-2 here unless they
                        # ride the attention stream via vsched
                        for ch in range(1 if v_in_stream else NCH - 1):
                            for ts in range(TC // P):
                                vjob0(ch * (TC // P) + ts)
                                u = next(pre_iter, None)
                                if u is not None:
                                    emit_s_unit(u)
                        # leftover pre-run units (if the group pace ran out)
                        for u in pre_iter:
                            emit_s_unit(u)

                    # ------- Phase C/D: attention + out-proj interleave ------
                    with (
                        tc.tile_pool(name="po", bufs=4, space="PSUM")
                        as po_psum,
                        tc.tile_pool(name="rcp", bufs=6) as rcp_pool,
                        tc.tile_pool(name="rcb", bufs=6) as rcb_pool,
                        tc.tile_pool(name="tmpn", bufs=3) as tmpn_pool,
                        tc.tile_pool(name="osb", bufs=4) as osb_pool,
                    ):
                        jobs = []   # pending out-projection (tt, chv, push#)
                        jid = [0]

                        def emit_job(pool=None):
                            tt, chv, _ = jobs.pop(0)
                            if pool is None:
                                pf_t = ps_psum.tile(
                                    [P, 2, 512], F32, tag="ps",
                                    name=f"pf_{_it}_{jid[0]}")
                                pf = pf_t[:, 0, :]
                            else:
                                pf_t = po_psum.tile(
                                    [P, 512], F32, tag="po",
                                    name=f"pf_{_it}_{jid[0]}")
                                pf = pf_t[:, 0:512]
                            jid[0] += 1
                            tail = pool is not None
                            if tail:
                                # seed PSUM with the bias (ones-row x
                                # bias-row) so the evacuation is a pure copy
                                # on the then-idle ACT engine
                                nc.tensor.matmul(
                                    pf, lhsT=trimask[0:1, 0:P],
                                    rhs=bout_row[:,
                                                 chv * 512:(chv + 1) * 512],
                                    start=True, stop=False)
                            for ko in range(CO):
                                nc.tensor.matmul(
                                    pf,
                                    lhsT=chunkT[:, ko, tt * P:(tt + 1) * P],
                                    rhs=wr_sb[:, ko,
                                              chv * 512:(chv + 1) * 512],
                                    start=False if tail else (ko == 0),
                                    stop=(ko == CO - 1))
                            osb = osb_pool.tile([P, 512], F32, tag="osb")
                            if tail:
                                nc.vector.tensor_copy(osb, pf)
                            else:
                                nc.vector.tensor_tensor(
                                    osb, pf,
                                    bout_b[:, chv * 512:(chv + 1) * 512],
                                    mybir.AluOpType.add)
                            nc.sync.dma_start(
                                out_d[tt * P:(tt + 1) * P,
                                      chv * 512:(chv + 1) * 512], osb)

                        def emit_norm(pi, h2, wi, gq0, pot):
                            # denominator row lives at partition D(=64);
                            # broadcast via a DRAM roundtrip: dscr write on
                            # SP, broadcast read on Pool SWDGE (last pair on
                            # SP too - lower latency, it gates out-proj)
                            rcp = rcp_pool.tile([P, 512], F32, tag="rcp")
                            nc.vector.reciprocal(rcp[D:D + 1, :],
                                                 pot[D:D + 1, :])
                            dscr = dram_pool.tile(
                                [1, 512], F32,
                                name=f"dscr_{_it}_{pi}_{h2}_{gq0}")
                            nc.sync.dma_start(dscr, rcp[D:D + 1, :])
                            rcb = rcb_pool.tile([D, 512], F32, tag="rcb")
                            eng = nc.sync if pi == NP - 1 else nc.gpsimd
                            eng.dma_start(rcb, dscr.to_broadcast((D, 512)))
                            if h2 == 0:
                                nc.vector.tensor_tensor(
                                    chunkT[0:D, pi, gq0:gq0 + 512],
                                    pot[0:D, :], rcb, mybir.AluOpType.mult)
                            else:
                                tm = tmpn_pool.tile([D, 512], F16,
                                                    tag="tmpn")
                                nc.vector.tensor_tensor(
                                    tm, pot[0:D, :], rcb,
                                    mybir.AluOpType.mult)
                                nc.gpsimd.tensor_copy(
                                    chunkT[D:2 * D, pi, gq0:gq0 + 512], tm)

                        pos = {}    # (c0, pi) -> {(h2, wi): po tile}
                        uidx = [0]  # current unit index (job age gating)

                        def emit_av(u):
                            c0, pi, avkt = u
                            qlo = c0 * QC
                            aqs = max(qlo, avkt * P)
                            po = pos[c0, pi]
                            # odd head first at the pair's last tile so its
                            # cross-partition copy starts earliest
                            h2s = (1, 0) if avkt == 8 * c0 + 7 else (0, 1)
                            for h2 in h2s:
                                h = 2 * pi + h2
                                for wi in range(QC // 512):
                                    gw = (QC // 512) * c0 + wi
                                    gq0 = qlo + wi * 512
                                    if avkt * P >= gq0 + 512:
                                        continue
                                    kt_last = 4 * (gw + 1) - 1
                                    a = max(0, avkt * P - gq0)
                                    nc.tensor.matmul(
                                        po[h2, wi][0:D + 1, a:512],
                                        lhsT=vt[:, avkt, h, :],
                                        rhs=ptts[u][:, h2,
                                                    gq0 + a - aqs:
                                                    gq0 + 512 - aqs],
                                        start=(avkt == 0),
                                        stop=(avkt == kt_last))
                                    if avkt == kt_last:
                                        emit_norm(pi, h2, wi, gq0,
                                                  po[h2, wi])
                                        if pi == NP - 1 and h2 == h2s[-1]:
                                            for tt in range(gq0 // P,
                                                            (gq0 + 512)
                                                            // P):
                                                jobs.append((tt, 0,
                                                             uidx[0]))
                                                jobs.append((tt, 1,
                                                             uidx[0]))
                            del ptts[u]

                        # flat software pipeline over (chunk, pair, key-tile)
                        # units with a constant lag between the S/exp cursor
                        # and the AV cursor (the pre-run supplies the initial
                        # offset), so ACT-heavy and PE-heavy stretches smooth
                        # out over a lag-sized window
                        units = [(c0, pi, kt)
                                 for c0 in range(NQC)
                                 for pi in range(NP)
                                 for kt in range((c0 * QC + QC) // P)]
                        # V chunks 1-3 keyed by the unit index they are
                        # emitted at; deadlines: vt[4..7] first read at
                        # unit 12, vt[8..15] only by c0=1 units (>= 32)
                        vsched = ({2: 4, 4: 5, 6: 6, 8: 7,
                                   14: 8, 18: 9, 22: 10, 26: 11,
                                   31: 12, 35: 13, 39: 14, 43: 15}
                                  if v_in_stream else
                                  {4: 12, 12: 13, 20: 14, 28: 15})

                        def emit_vjob(kt_idx):
                            ch, ts = kt_idx // (TC // P), kt_idx % (TC // P)
                            pv_t = ps_psum.tile([P, 2, 512], F32, tag="ps",
                                                name=f"pv_{_it}_{kt_idx}")
                            pv = pv_t[:, 0, :]
                            for ko in range(KO):
                                nc.tensor.matmul(
                                    pv,
                                    lhsT=x16[:, ko,
                                             ch * TC + ts * P:
                                             ch * TC + (ts + 1) * P],
                                    rhs=wv_sb[:, ko, :],
                                    start=(ko == 0), stop=(ko == KO - 1))
                            nc.vector.tensor_copy(
                                vt[:, kt_idx, :, 0:D],
                                pv.rearrange("p (h d) -> p h d", d=D))

                        scur = [len(pre_units)]

                        def feed_s(ai, budget):
                            while (budget > 0 and scur[0] < len(units)
                                   and scur[0] - ai <= lagm):
                                emit_s_unit(units[scur[0]])
                                scur[0] += 1
                                budget -= 1

                        for ai, ua in enumerate(units):
                            feed_s(ai, 2)
                            if ua[2] == 0:
                                c0, pi = ua[0], ua[1]
                                pos[c0, pi] = {
                                    (h2, wi): po_psum.tile(
                                        [P, 512], F32, tag="po",
                                        name=(f"po_{_it}_{c0}_{pi}"
                                              f"_{h2}_{wi}"))
                                    for h2 in range(2)
                                    for wi in range(QC // 512)}
                            emit_av(ua)
                            # drain out-proj tiles one at a time (bursts
                            # stall the exp stream); only jobs aged past
                            # the ~7us normalization chain.
                            if uidx[0] in vsched:
                                emit_vjob(vsched[uidx[0]])
                            if scur[0] >= len(units):
                                # S cursor exhausted: the exp stream can no
                                # longer stall on PSUM-ring pressure, so
                                # drain the job backlog aggressively
                                for _ in range(3):
                                    if jobs and uidx[0] - jobs[0][2] >= 2:
                                        emit_job()
                            elif (jobs and uidx[0] % drain_every == 0
                                    and uidx[0] - jobs[0][2] >= drain_age):
                                # po pool, not the S-unit ring: a pf tile in
                                # the ps ring stalls the exp pipeline
                                emit_job(pool='po')
                            uidx[0] += 1
                        # final drain: po pool is idle too, alternate pf
                        # tiles between both PSUM pools for more overlap
                        while jobs:
                            emit_job(pool='po' if jid[0] % 2 else None)

    if split:
        split_waits(nc)
    return nc


def make_in_maps(x, Wqkv, bqkv, Wout, bout, n_cores=8):
    """Slice full inputs into per-core input maps (host pre-casts the
    fp8/fp16 operands and pre-transposes x)."""
    import ml_dtypes
    E4 = ml_dtypes.float8_e4m3
    x = np.ascontiguousarray(np.asarray(x, dtype=np.float32))
    Wqkv = np.asarray(Wqkv, dtype=np.float32)
    bqkv = np.asarray(bqkv, dtype=np.float32)
    Wout = np.ascontiguousarray(np.asarray(Wout, dtype=np.float32))
    bout = np.asarray(bout, dtype=np.float32)
    C = x.shape[2]
    CL = C // 2
    bv_full = bqkv[2 * C:3 * C]
    bout_eff = (bout + bv_full @ Wout).astype(np.float32)
    zeros_b = np.zeros_like(bout_eff)
    in_maps = []
    # per-128-col-block permutation: position 64*h2 + 2*r + j takes the
    # column 64*h2 + 32*j + r (head h2-half, d = 32*j + r), so the psum
    # staging comes out j-interleaved for the 2-DMA slot scatter
    h2g, rg, jg = np.meshgrid(np.arange(2), np.arange(32), np.arange(2),
                              indexing='ij')
    perm128 = (64 * h2g + 32 * jg + rg).reshape(-1)
    permQK = np.concatenate([128 * blk + perm128 for blk in range(8)])
    for core in range(n_cores):
        b, g = core // 2, core % 2
        xt = np.ascontiguousarray(x[b].T)
        wqk = np.ascontiguousarray(np.concatenate(
            [Wqkv[:, g * CL:(g + 1) * CL],
             Wqkv[:, C + g * CL:C + (g + 1) * CL]], axis=1))[:, permQK]
        in_maps.append({
            "x8": xt.astype(E4),
            "x": xt.astype(np.float16),
            "wqk8": (wqk * WSC).astype(E4),
            "wv": np.ascontiguousarray(
                Wqkv[:, 2 * C + g * CL:2 * C + (g + 1) * CL]).astype(
                    np.float16),
            "bq": np.ascontiguousarray(
                (bqkv[g * CL:(g + 1) * CL] * WSC)[permQK[:CL]]).astype(
                    np.float32),
            "wout": np.ascontiguousarray(
                Wout[g * CL:(g + 1) * CL, :].astype(np.float16)),
            "bout": bout_eff if g == 0 else zeros_b,
        })
    return in_maps


_NC_CACHE = {}


def _get_nc(T=2048):
    if T not in _NC_CACHE:
        _NC_CACHE[T] = build_nc(T=T)
    return _NC_CACHE[T]


def kernel(x, mask, Wqkv, bqkv, Wout, bout, _trace=False, _trace_kwargs=None):
    from concourse.bass_utils import run_bass_kernel_spmd

    x = np.asarray(x)
    B, T, C = x.shape
    nc = _get_nc(T=T)
    in_maps = make_in_maps(x, Wqkv, bqkv, Wout, bout)
    kw = {}
    if _trace:
        kw = dict(trace=True, **(_trace_kwargs or {}))
    res = run_bass_kernel_spmd(nc, in_maps, core_ids=list(range(8)), **kw)
    out = np.zeros((B, T, C), np.float32)
    for core in range(8):
        out[core // 2] += res.results[core]["out"]
    if _trace:
        return out, res
    return out
